# revision 1
# baseline (speedup 1.0000x reference)
"""BitLinear (ternary-weight + int8-activation quantized linear) on 8 Trainium2
NeuronCores, column-parallel over out_features.

Contract: kernel(x, weight) with x (2, 2048, 4096) f32, weight (16384, 4096) f32
returns (2, 2048, 16384) f32 — the full unsharded output.

Strategy (v2 — restructured from the 1.40 ms baseline)
------------------------------------------------------
- Shard weight rows (out_features) 8 ways; replicate x. The per-core weight
  slice is passed HOST-TRANSPOSED as wT [D_IN, OPC] f32, so the device never
  transposes weights: a ternarized [128, 512] chunk of wT IS the matmul-ready
  moving operand slice w8[:, k, og]. This deletes the baseline's bf16 xbar
  transpose + gpsimd fp8 copy from the weight path entirely.
- The quantized GEMM is exact integer math: |x_q| <= 127 fits bf16 exactly and
  ternary weights {-1,0,+1} fit fp8e4m3 exactly, so bf16(stationary x) x
  fp8(moving w) matmuls with fp32 PSUM accumulation reproduce it bit-exactly;
  all scales fold into an fp32 epilogue (gamma * scale_w / 127 per token).
- Critical path: pass-1 abs-sums the fp32 wT slice at full DMA rate (scalar
  ring) while x tiles 0-1 prep on the sync ring; the partition reduce uses
  gpsimd.partition_all_reduce (not the slow C-axis reduce), the 4-byte
  AllReduce result is partition-broadcast on-chip (no DRAM round trips).
  og0's pass-2 re-reads are enqueued on the scalar ring right behind pass-1,
  so they stream during the AllReduce's dead window; ternarization (3 DVE ops
  straight into the resident fp8 w8) fires the moment the scale lands.
- Ramp: og-outer over x tiles 0-2 while og1..3 ternarize; x loads for t>=2
  wait for og0's DMA so weight re-reads keep bandwidth priority. Steady state:
  t-outer, k-outer/og-inner matmuls (stationary xqT[:,k,:] reused across the 4
  output groups), ScalarE applies the fp32 epilogue on PSUM->SBUF, sync-ring
  DMA streams results out.
"""

import sys

sys.path.insert(0, "/opt/trn_rl_repo")

import numpy as np

import concourse.bass as bass
import concourse.bass_isa as bass_isa
import concourse.mybir as mybir
import concourse.tile as tile
import bass_rust
from concourse.bass_utils import run_bass_kernel_spmd

F32 = mybir.dt.float32
BF16 = mybir.dt.bfloat16
FP8 = mybir.dt.float8e4
CMAGIC = 12582912.0  # 2^23 + 2^22: (v + C) - C == round-half-even(v), |v| < 2^22
EPS = 1e-8

N_CORES = 8
B, T, D_IN, D_OUT = 2, 2048, 4096, 16384
TOK = B * T                      # 4096 tokens
OPC = D_OUT // N_CORES           # 2048 out features per core
NTOK = TOK // 128                # 32 token tiles
ND = D_IN // 128                 # 32 contraction tiles
NOG = OPC // 512                 # 4 output groups
DH = D_IN // 2                   # 2048 x staging width
NDH = DH // 128                  # 16 d-tiles per half
RAMP_TILES = 2                   # token tiles covered by the k-outer ramp
NPIN = 2                         # pass-1 chunks pinned in SBUF for pass-2
XAHEAD = 2                       # steady-state x-prep lookahead


def _split_multi_waits(nc):
    """This container's walrus build rejects >1 sync wait per instruction, but
    Tile emits multi-wait instructions. Move extra waits onto preceding
    single-wait NoOps on the same engine (identical blocking semantics)."""
    wid = 0
    for f in nc.m.functions:
        for blk in f.blocks:
            insts = list(blk.instructions)
            new = []
            changed = False
            for inst in insts:
                si = inst.sync_info
                if si is not None and len(si.on_wait) > 1:
                    waits = list(si.on_wait)
                    for w in waits[:-1]:
                        nop = mybir.InstNoOp(name=f"WSPLIT-{wid}", ins=[], outs=[])
                        wid += 1
                        nop.engine = inst.engine
                        nop.sync_info = bass_rust.SyncInfo(on_wait=[w], on_update=[])
                        new.append(nop)
                    inst.sync_info = bass_rust.SyncInfo(
                        on_wait=[waits[-1]], on_update=list(si.on_update)
                    )
                    changed = True
                new.append(inst)
            if changed:
                blk.instructions = new


def build_bitlinear_nc():
    nc = bass.Bass("TRN2", target_bir_lowering=False, debug=False,
                   num_devices=N_CORES)
    x_d = nc.dram_tensor("x", [TOK, D_IN], F32, kind="ExternalInput")
    wT_d = nc.dram_tensor("wT", [D_IN, OPC], F32, kind="ExternalInput")
    out_d = nc.dram_tensor("out", [TOK, OPC], F32, kind="ExternalOutput")
    cc_buf = nc.dram_tensor("cc_buf", [1, 1], F32)

    with tile.TileContext(nc, trace_sim=False) as tc:
        with (
            tc.tile_pool(name="w8p", bufs=1) as w8_pool,
            tc.tile_pool(name="wpin", bufs=1) as wpin_pool,     # pinned pass-1
            tc.tile_pool(name="w32", bufs=4) as w32_pool,       # streaming w
            tc.tile_pool(name="wtw", bufs=2) as wtw_pool,       # magic-add f32
            tc.tile_pool(name="wmid", bufs=1) as wmid_pool,     # tern bf16
            tc.tile_pool(name="x32", bufs=4) as x32_pool,
            tc.tile_pool(name="xt1", bufs=2) as xt1_pool,
            tc.tile_pool(name="xq16", bufs=2) as xq16_pool,
            tc.tile_pool(name="xqT", bufs=4) as xqT_pool,
            tc.tile_pool(name="outs", bufs=2) as outs_pool,
            tc.tile_pool(name="small", bufs=1) as small,
            tc.tile_pool(name="psum", bufs=2, space="PSUM") as psum_pool,
        ):
            # resident ternary weight, matmul-ready: [d % 128, d // 128, o]
            w8 = w8_pool.tile([128, ND, OPC], FP8, tag="w8", name="w8")
            partials = small.tile([128, ND], F32)
            cmag = small.tile([128, 1], F32)
            nc.gpsimd.memset(cmag[:], CMAGIC)

            # x t0/t1 loads lead the scalar ring: their ~4MB steals a bit of
            # pass-1 bandwidth but the whole quant chain then hides under
            # pass-1 + the collective.
            xh01 = {}
            for t in range(2):
                xh01[t] = [x32_pool.tile([128, DH], F32, tag="x32",
                                         name=f"x_{t}_{h}") for h in range(2)]
                for h in range(2):
                    nc.scalar.dma_start(
                        xh01[t][h][:],
                        x_d[t * 128:(t + 1) * 128, h * DH:(h + 1) * DH])

            # ---- pass 1: abs-sum of the fp32 wT slice (scalar ring) ----
            # k=0..NPIN-1 are read LAST so their fp32 chunks stay pinned in
            # the pool for pass-2 to ternarize instantly once the scale lands.
            pinned = {}
            for i, k in enumerate(list(range(NPIN, ND)) + list(range(NPIN))):
                if k < NPIN:
                    wchunk = wpin_pool.tile([128, OPC], F32, tag=f"wpin{k}",
                                            name=f"wpin_{k}")
                    pinned[k] = wchunk
                else:
                    wchunk = w32_pool.tile([128, OPC], F32, tag="w32",
                                           name=f"w32_{k}")
                # scalar ring only: the SP ring issues DMAs ~10x slower (its
                # queue is interleaved with Tile's semaphore bookkeeping)
                nc.scalar.dma_start(wchunk[:], wT_d[k * 128:(k + 1) * 128, :])
                nc.vector.tensor_reduce(
                    partials[:, k:k + 1], wchunk[:],
                    axis=mybir.AxisListType.X,
                    op=mybir.AluOpType.add, apply_absolute_value=True)

            # partials -> one scalar -> AllReduce across the 8 cores.
            # Cross-partition reduce via a ones-matmul: the PE is idle here
            # (no ternary weights exist yet) and it beats the gpsimd C-axis
            # reduce by ~10us of trigger latency.
            psum1 = small.tile([128, 1], F32)
            p1red = nc.vector.tensor_reduce(psum1[:], partials[:],
                                            axis=mybir.AxisListType.X,
                                            op=mybir.AluOpType.add)
            ones = small.tile([128, 1], F32)
            nc.gpsimd.memset(ones[:], 1.0)
            lps = psum_pool.tile([1, 1], F32, tag="acc0", name="lsum_ps")
            nc.tensor.matmul(lps[:], ones[:], psum1[:], start=True, stop=True)
            lsum = small.tile([1, 1], F32)
            nc.scalar.activation(lsum[:], lps[:],
                                 mybir.ActivationFunctionType.Copy,
                                 bias=0.0, scale=1.0)
            nc.scalar.dma_start(cc_buf[:], lsum[:])
            nc.gpsimd.collective_compute(
                "AllReduce", mybir.AluOpType.add,
                replica_groups=[list(range(N_CORES))],
                ins=[cc_buf[:]], outs=[cc_buf[:]])

            # The scale broadcast + per-lane scale math are emitted LATER
            # (in the schedule, after the early x preps) so the CC wait never
            # head-of-line blocks the sync/DVE queues doing x work.
            scb = small.tile([128, 2], F32)
            rsw_b = scb[:, 0:1]
            sw127_b = scb[:, 1:2]

            def emit_scale_math():
                gsb = small.tile([128, 1], F32)
                nc.scalar.dma_start(gsb[:], cc_buf[:].partition_broadcast(128))
                nc.vector.tensor_scalar(scb[:, 0:1], gsb[:],
                                        1.0 / (D_OUT * D_IN), EPS,
                                        op0=mybir.AluOpType.mult,
                                        op1=mybir.AluOpType.add)
                nc.vector.reciprocal(scb[:, 0:1], scb[:, 0:1])
                nc.vector.tensor_scalar_mul(scb[:, 1:2], gsb[:],
                                            1.0 / (D_OUT * D_IN * 127.0))

            # ---- pass 2: k-major full-width re-read (8KB DMA lines; the og-
            # column variant measured only ~78 GB/s) + ACT/DVE ternarize ----
            def tern_k(k):
                if k in pinned:
                    wc = pinned[k]
                else:
                    # sync ring: a WAR-stalled re-read submit must not block
                    # the ACT queue (whose head waits on the global scale)
                    wc = w32_pool.tile([128, OPC], F32, tag="w32",
                                       name=f"w32b_{k}")
                    nc.sync.dma_start(wc[:], wT_d[k * 128:(k + 1) * 128, :])
                for h in range(2):
                    sl = slice(h * 1024, (h + 1) * 1024)
                    tw = wtw_pool.tile([128, 1024], F32, tag="wtw")
                    nc.scalar.activation(tw[:], wc[:, sl],
                                         mybir.ActivationFunctionType.Identity,
                                         bias=cmag[:], scale=rsw_b)
                    tm = wmid_pool.tile([128, 1024], BF16, tag="wmid")
                    nc.vector.tensor_scalar(tm[:], tw[:], -CMAGIC, -1.0,
                                            op0=mybir.AluOpType.add,
                                            op1=mybir.AluOpType.max)
                    nc.vector.tensor_scalar_min(w8[:, k, sl], tm[:], 1.0)

            # ---- x pipeline ----
            xqTs = {}
            evecs = {}

            gams = {}

            def emit_evec(t):
                evec = small.tile([128, 1], F32, tag=f"ev{t % 6}", name=f"ev_{t}")
                nc.vector.tensor_tensor(out=evec[:], in0=gams[t], in1=sw127_b,
                                        op=mybir.AluOpType.mult)
                evecs[t] = evec

            def x_load(t):
                # scalar ring: ring order serializes these transfers against
                # pass-1's submits without any explicit dependency
                xh = []
                for h in range(2):
                    xt = x32_pool.tile([128, DH], F32, tag="x32", name=f"x_{t}_{h}")
                    nc.scalar.dma_start(
                        xt[:], x_d[t * 128:(t + 1) * 128, h * DH:(h + 1) * DH])
                    xh.append(xt)
                return xh

            def x_compute(t, xh, defer_evec=False):
                gpart = small.tile([128, 2], F32, tag=f"gp{t % 6}",
                                   name=f"gp_{t}")
                for h in range(2):
                    nc.vector.tensor_reduce(gpart[:, h:h + 1], xh[h][:],
                                            axis=mybir.AxisListType.X,
                                            op=mybir.AluOpType.max,
                                            apply_absolute_value=True)
                gv = small.tile([128, 2], F32, tag=f"gv{t % 6}", name=f"gv_{t}")
                gam, qs = gv[:, 0:1], gv[:, 1:2]
                nc.vector.tensor_reduce(gam, gpart[:], axis=mybir.AxisListType.X,
                                        op=mybir.AluOpType.max)
                nc.vector.tensor_scalar_add(qs, gam, EPS)
                nc.vector.reciprocal(qs, qs)
                nc.vector.tensor_scalar_mul(qs, qs, 127.0)
                gams[t] = gam
                if not defer_evec:
                    emit_evec(t)

                # steady-state transposes ride the (then-idle) scalar ring so
                # they never head-of-line block x loads / out stores; ramp-era
                # tiles stay on sync while the scalar ring streams pass-2.
                teng = nc.sync if t < 6 else nc.scalar
                xqT = xqT_pool.tile([128, ND, 128], BF16, tag="xqT", name=f"xqT_{t}")
                for h in range(2):
                    xq16 = xq16_pool.tile([128, DH], BF16, tag="xq16")
                    for q in range(2):
                        sl = slice(q * 1024, (q + 1) * 1024)
                        x1 = xt1_pool.tile([128, 1024], F32, tag="xt1")
                        nc.scalar.activation(x1[:], xh[h][:, sl],
                                             mybir.ActivationFunctionType.Identity,
                                             bias=cmag[:], scale=qs)
                        nc.vector.tensor_scalar_add(xq16[:, sl], x1[:], -CMAGIC)
                    teng.dma_start_transpose(
                        out=xqT[:, h * NDH:(h + 1) * NDH, :], in_=xq16[:])
                xqTs[t] = xqT

            def epilogue(t, og, acc):
                ot = outs_pool.tile([128, 512], F32, tag="outs")
                nc.scalar.activation(ot[:], acc[:],
                                     mybir.ActivationFunctionType.Copy,
                                     bias=0.0, scale=evecs[t][:])
                nc.sync.dma_start(
                    out_d[t * 128:(t + 1) * 128, og * 512:(og + 1) * 512], ot[:])

            def mm_tile(t):
                accs = [psum_pool.tile([128, 512], F32, tag=f"acc{og}",
                                       name=f"acc_{t}_{og}")
                        for og in range(NOG)]
                xqT = xqTs[t]
                for k in range(ND):
                    for og in range(NOG):
                        nc.tensor.matmul(
                            accs[og][:], xqT[:, k, :],
                            w8[:, k, og * 512:(og + 1) * 512],
                            start=(k == 0), stop=(k == ND - 1))
                for og in range(NOG):
                    epilogue(t, og, accs[og])

            # ---- schedule ----
            x_compute(0, xh01[0], defer_evec=True)
            x_compute(1, xh01[1], defer_evec=True)
            x_compute(2, x_load(2), defer_evec=True)
            x_compute(3, x_load(3), defer_evec=True)
            emit_scale_math()
            for t in range(4):
                emit_evec(t)
            # ramp: k-outer over 8 PSUM-resident groups (t0..1 x og0..3) so
            # the PE consumes every ternarized k-chunk the moment it lands.
            # Pinned chunks (0,1,2,31) ternarize instantly after the scale.
            ks_ramp = list(range(NPIN)) + list(range(NPIN, ND))
            groups = [(t, og) for t in range(RAMP_TILES) for og in range(NOG)]
            accs = {}
            for t, og in groups:
                accs[(t, og)] = psum_pool.tile([128, 512], F32, tag=f"acc{og}",
                                               name=f"acc_{t}_{og}")
            for idx, k in enumerate(ks_ramp):
                tern_k(k)
                for t, og in groups:
                    nc.tensor.matmul(accs[(t, og)][:], xqTs[t][:, k, :],
                                     w8[:, k, og * 512:(og + 1) * 512],
                                     start=(idx == 0), stop=(idx == ND - 1))
            for t, og in groups:
                epilogue(t, og, accs[(t, og)])
            # steady state: t-outer, k-outer/og-inner
            for t in range(RAMP_TILES, NTOK):
                ta = t + XAHEAD
                if RAMP_TILES + 1 < ta < NTOK:
                    x_compute(ta, x_load(ta))
                mm_tile(t)

    _split_multi_waits(nc)
    return nc


_NC_CACHE = None


def kernel(x: np.ndarray, weight: np.ndarray, _want_profile=False, **_kw):
    global _NC_CACHE
    assert x.shape == (B, T, D_IN) and weight.shape == (D_OUT, D_IN)
    x_flat = np.ascontiguousarray(x.reshape(TOK, D_IN), dtype=np.float32)
    w = np.ascontiguousarray(weight, dtype=np.float32)

    if _NC_CACHE is None:
        _NC_CACHE = build_bitlinear_nc()
    nc = _NC_CACHE

    in_maps = [
        {"x": x_flat,
         "wT": np.ascontiguousarray(w[c * OPC:(c + 1) * OPC, :].T)}
        for c in range(N_CORES)
    ]
    res = run_bass_kernel_spmd(nc, in_maps, list(range(N_CORES)),
                               trace=bool(_want_profile))
    out = np.concatenate([res.results[c]["out"] for c in range(N_CORES)], axis=1)
    out = out.reshape(B, T, D_OUT)
    if _want_profile:
        return out, res
    return out

